# revision 1
# baseline (speedup 1.0000x reference)
"""Trainium2 Bass kernel for nn_CrossModalFusionCore (B=8, S=1024, D=1024, H=16).

Structure exploited: in the reference, K/V of the first cross-attention come
from a per-batch vector broadcast across the sequence (softmax over identical
scores -> uniform -> output == V vector), and the queries of the second
cross-attention are all identical (one attention distribution per head per
batch). Hence the entire output is constant across the sequence dimension,
and per batch the real tensor work is:

  scores[s,h] = (seq_b[s] . M_b[:,h] + c_b[h]) / 8   (M_b = Wk_h^T q_h)
  attn = softmax_s(scores);  w_b = seq_b^T @ attn                [D,H]
  ctx[h-block] = Wv_h @ w_b[:,h] + bv_h;  ga = ow @ ctx + ob
  sa = ow @ (Wv g_b + bv) + ob            (host-precomputable)
  gate = sigmoid(gate_w @ [sa;ga] + gate_b)
  x = proj_w @ [sa;ga] + proj_b + gate*sa + (1-gate)*ga
  out_b[s,:] = LayerNorm(x) for all s

Distribution: data-parallel over batch (core b owns seq_b attention) +
tensor-parallel epilogue (core j owns a 128-wide slice of the ctx dimension,
i.e. heads 2j,2j+1). Two collectives: an AllToAll that routes each batch's
per-head attention reads w_b to the core owning those heads, and an AllReduce
that sums the input-sharded epilogue partials. Weight-only compositions
(G=gate_w@ow, P=proj_w@ow and the per-batch vectors q_g, v_g, M, sa, gl0,
pl0) are folded on the host, so the device only loads ~5MB/core.
"""
import numpy as np
import ml_dtypes
from contextlib import ExitStack

import concourse.bass as bass
import concourse.tile as tile
from concourse import bacc, mybir
from concourse.bass_utils import run_bass_kernel_spmd
from concourse.masks import make_identity

B, S, D, H = 8, 1024, 1024, 16
HD = D // H
NCORES = 8
EPS = 1e-5
BF = mybir.dt.bfloat16
F32 = mybir.dt.float32

# test.py hooks
TRACE = False
TRACE_CORES = None
LAST_RESULT = None

_cache = {}


def _body(ctx, tc, io):
    nc = tc.nc
    const = ctx.enter_context(tc.tile_pool(name="const", bufs=1))
    work = ctx.enter_context(tc.tile_pool(name="work", bufs=1))
    psum = ctx.enter_context(tc.tile_pool(name="psum", bufs=3, space="PSUM"))
    dram = ctx.enter_context(tc.tile_pool(name="dram", bufs=1, space="DRAM"))
    rg = [list(range(NCORES))]

    # ---- small loads needed first ----
    msc_sb = const.tile([128, 8, H], BF)
    nc.sync.dma_start(out=msc_sb[:, :, :], in_=io["msc"])
    cb8_sb = const.tile([H, 1], F32)
    nc.scalar.dma_start(out=cb8_sb[:, :], in_=io["cb8"])
    ident = const.tile([128, 128], BF)
    make_identity(nc, ident)

    # ---- big seq loads: 2 DMAs each, split across both HWDGE engines ----
    seqT_sb = const.tile([128, 8, S], BF)  # [d-part, d-chunk, s]
    seqN_sb = const.tile([128, 8, D], BF)  # [s-part, s-chunk, d]
    for c in range(8):
        nc.sync.dma_start(out=seqT_sb[:, c, :],
                          in_=io["seqT"][c * 128:(c + 1) * 128, :])
        nc.scalar.dma_start(out=seqN_sb[:, c, :],
                          in_=io["seqN"][c * 128:(c + 1) * 128, :])

    # ---- scores^T = M^T @ seq^T, then exp((scores + c)/8) fused on ACT ----
    scope_p1 = nc.named_scope("p1_attn"); scope_p1.__enter__()
    expT = work.tile([H, S], F32)
    for half in range(2):
        ps = psum.tile([128, 512], F32, tag="mm", bufs=4, name=f"ps{half}")[0:H, :]
        for c in range(8):
            nc.tensor.matmul(ps[:, :], msc_sb[:, c, :],
                             seqT_sb[:, c, 512 * half:512 * (half + 1)],
                             start=(c == 0), stop=(c == 7))
        nc.scalar.activation(out=expT[:, 512 * half:512 * (half + 1)],
                             in_=ps[:, :],
                             func=mybir.ActivationFunctionType.Exp,
                             bias=cb8_sb[:, :], scale=0.125)

    # ---- softmax normalize; cast to bf16 ----
    ssum = work.tile([H, 1], F32)
    nc.vector.reduce_sum(out=ssum[:, :], in_=expT[:, :], axis=mybir.AxisListType.X)
    rsum = work.tile([H, 1], F32)
    nc.vector.reciprocal(out=rsum[:, :], in_=ssum[:, :])
    attnT = work.tile([H, S], BF)
    nc.vector.tensor_scalar_mul(out=attnT[:, :], in0=expT[:, :], scalar1=rsum[:, :])

    # ---- transpose attn to [s-part, (c,h)] in one PSUM tile ----
    tpa = psum.tile([128, 512], BF, tag="tp", bufs=2, name="tpa")[:, 0:128]
    for c in range(8):
        nc.tensor.transpose(tpa[:, c * H:(c + 1) * H],
                            attnT[:, c * 128:(c + 1) * 128], ident[0:H, 0:H])
    attn_sb = work.tile([128, 128], BF)
    nc.vector.tensor_copy(out=attn_sb[:, :], in_=tpa[:, :])

    # ---- w^T = attn^T @ seq  -> [H, D] (bf16 for the AllToAll) ----
    wT = work.tile([H, D], BF)
    for half in range(2):
        psw = psum.tile([128, 512], F32, tag="mm", bufs=4, name=f"psw{half}")[0:H, :]
        for c in range(8):
            nc.tensor.matmul(psw[:, :], attn_sb[:, c * H:(c + 1) * H],
                             seqN_sb[:, c, 512 * half:512 * (half + 1)],
                             start=(c == 0), stop=(c == 7))
        nc.vector.tensor_copy(out=wT[:, 512 * half:512 * (half + 1)], in_=psw[:, :])

    scope_p1.__exit__(None, None, None)
    # ---- AllToAll: row-pair (2j, 2j+1) -> core j; receive rows (2b+hh) ----
    scope_p2 = nc.named_scope("p2_a2a"); scope_p2.__enter__()
    a2a_in = dram.tile([H, D], BF)
    nc.sync.dma_start(out=a2a_in[:, :], in_=wT[:, :])
    a2a_out = dram.tile([H, D], BF)
    nc.gpsimd.collective_compute("AllToAll", mybir.AluOpType.bypass,
                                 replica_groups=rg,
                                 ins=[a2a_in.opt()], outs=[a2a_out.opt()])

    # ---- deferred loads (overlap with attention/collective) ----
    wvT_sb = const.tile([128, 8, 128], BF)
    nc.scalar.dma_start(out=wvT_sb[:, :, :], in_=io["wvT"])
    bvj_sb = const.tile([128, 1], F32)
    nc.scalar.dma_start(out=bvj_sb[:, :], in_=io["bvj"])
    w3_sb = const.tile([128, 3, D], BF)   # owT, g2T, p2T packed
    for i in range(3):
        nc.sync.dma_start(out=w3_sb[:, i, :], in_=io["w3T"][:, i, :])
    vec_sb = const.tile([64, 5, 128], F32)   # sa0, gl0, pl0p, lgr, lbr packed
    for i in range(5):
        nc.scalar.dma_start(out=vec_sb[:, i, :], in_=io["vec5"][:, i, :])
    obd_sb = const.tile([64, B], F32)    # blockdiag ones: [p, b] = (p//8 == b)
    nc.sync.dma_start(out=obd_sb[:, :], in_=io["obd"])
    obt_sb = const.tile([B, 64], F32)    # its transpose
    nc.scalar.dma_start(out=obt_sb[:, :], in_=io["obt"])
    selB_sb = const.tile([64, B, 128], F32)
    for i in range(2):
        nc.sync.dma_start(out=selB_sb[:, 4 * i:4 * (i + 1), :],
                          in_=io["selB"][:, 4 * i:4 * (i + 1), :])
    sel_sb = const.tile([64, B], F32)
    nc.sync.dma_start(out=sel_sb[:, :], in_=io["sel"])

    wr = work.tile([H, D], BF)
    nc.sync.dma_start(out=wr[:, 0:512], in_=a2a_out[:, 0:512])
    nc.sync.dma_start(out=wr[:, 512:1024], in_=a2a_out[:, 512:1024])

    scope_p2.__exit__(None, None, None)
    # ---- transpose received w to [d-part, (c -> (b,hh))] ----
    scope_p3 = nc.named_scope("p3_ctx"); scope_p3.__enter__()
    tpw = psum.tile([128, 512], BF, tag="tp", bufs=2, name="tpw")[:, 0:128]
    for c in range(8):
        nc.tensor.transpose(tpw[:, c * H:(c + 1) * H],
                            wr[:, c * 128:(c + 1) * 128], ident[0:H, 0:H])
    wD = work.tile([128, 128], BF)
    nc.vector.tensor_copy(out=wD[:, :], in_=tpw[:, :])

    # ---- ctx^T[c in slice_j, b] = Wv_h @ w_b_h ----
    ps_ctx = psum.tile([128, 512], F32, tag="ctx", bufs=1, name="ps_ctx")[:, 0:B]
    for hh in range(2):
        for c in range(8):
            rhs = wD[:, c * H:(c + 1) * H].rearrange(
                "p (b hh) -> p hh b", hh=2)[:, hh, :]
            nc.tensor.matmul(ps_ctx[hh * 64:(hh + 1) * 64, :],
                             wvT_sb[:, c, hh * 64:(hh + 1) * 64], rhs,
                             start=(c == 0), stop=(c == 7))
    ctxs = work.tile([128, B], F32)
    nc.vector.tensor_scalar_add(out=ctxs[:, :], in0=ps_ctx[:, :], scalar1=bvj_sb[:, :])
    ctxb = work.tile([128, B], BF)
    nc.vector.tensor_copy(out=ctxb[:, :], in_=ctxs[:, :])

    # ---- input-sharded epilogue partials: ga_p, gl_p, pl_p  [8, D] each ----
    ar_in = dram.tile([24, D], BF)
    for i in range(3):
        pt = work.tile([B, D], BF, name=f"pt{i}", tag="pt", bufs=2)
        for half in range(2):
            pp = psum.tile([128, 512], F32, tag="mm", bufs=4, name=f"pp{i}{half}")[0:B, :]
            nc.tensor.matmul(pp[:, :], ctxb[:, :],
                             w3_sb[:, i, 512 * half:512 * (half + 1)],
                             start=True, stop=True)
            nc.vector.tensor_copy(
                out=pt[:, 512 * half:512 * (half + 1)], in_=pp[:, :])
        nc.sync.dma_start(out=ar_in[8 * i:8 * (i + 1), :], in_=pt[:, :])
    scope_p3.__exit__(None, None, None)
    scope_p4 = nc.named_scope("p4_ar"); scope_p4.__enter__()
    ar_out = dram.tile([24, D], BF, addr_space="Shared")
    nc.gpsimd.collective_compute("AllReduce", mybir.AluOpType.add,
                                 replica_groups=rg,
                                 ins=[ar_in.opt()], outs=[ar_out.opt()])
    def ar_slice64(sect):
        a = ar_out[sect * 8:(sect + 1) * 8, :]
        return bass.AP(tensor=a.tensor, offset=a.offset,
                       ap=[[128, 64], [1, 128]])
    ars_ga = work.tile([64, 128], BF)
    nc.sync.dma_start(out=ars_ga[:, :], in_=ar_slice64(0))
    ars_gl = work.tile([64, 128], BF)
    nc.scalar.dma_start(out=ars_gl[:, :], in_=ar_slice64(1))
    ars_pl = work.tile([64, 128], BF)
    nc.sync.dma_start(out=ars_pl[:, :], in_=ar_slice64(2))

    scope_p4.__exit__(None, None, None)
    # ---- tail (ob folded on host: sa0 = sa-ob, pl0p = pl0+ob) ----
    # x = (pl0p + ars_pl) + ars_ga + gate*(sa0 - ars_ga);  gate = sig(gl0+ars_gl)
    scope_p5 = nc.named_scope("p5_tail"); scope_p5.__enter__()
    gl = work.tile([64, 128], F32)
    nc.vector.tensor_add(out=gl[:, :], in0=ars_gl[:, :], in1=vec_sb[:, 1, :])
    gate = work.tile([64, 128], F32)
    nc.scalar.activation(out=gate[:, :], in_=gl[:, :],
                         func=mybir.ActivationFunctionType.Sigmoid)
    d1 = work.tile([64, 128], F32)
    nc.vector.tensor_sub(out=d1[:, :], in0=vec_sb[:, 0, :], in1=ars_ga[:, :])
    gd = work.tile([64, 128], F32)
    nc.vector.tensor_mul(out=gd[:, :], in0=gate[:, :], in1=d1[:, :])
    t1 = work.tile([64, 128], F32)
    nc.vector.tensor_add(out=t1[:, :], in0=ars_pl[:, :], in1=ars_ga[:, :])
    t2 = work.tile([64, 128], F32)
    nc.vector.tensor_add(out=t2[:, :], in0=t1[:, :], in1=vec_sb[:, 2, :])
    x_ = work.tile([64, 128], F32)
    nc.vector.tensor_add(out=x_[:, :], in0=t2[:, :], in1=gd[:, :])

    # LN stats via blockdiag-ones matmul: per-batch sums over 8 partitions
    xsq = work.tile([64, 128], F32)
    nc.vector.tensor_mul(out=xsq[:, :], in0=x_[:, :], in1=x_[:, :])
    ps_st = psum.tile([128, 512], F32, tag="ctx", bufs=1, name="ps_st")[0:B, 0:256]
    nc.tensor.matmul(ps_st[:, 0:128], obd_sb[:, :], x_[:, :],
                     start=True, stop=True)
    nc.tensor.matmul(ps_st[:, 128:256], obd_sb[:, :], xsq[:, :],
                     start=True, stop=True)
    sums = work.tile([B, 2], F32)
    nc.vector.reduce_sum(out=sums[:, 0:1], in_=ps_st[:, 0:128],
                         axis=mybir.AxisListType.X)
    nc.vector.reduce_sum(out=sums[:, 1:2], in_=ps_st[:, 128:256],
                         axis=mybir.AxisListType.X)
    # mu = sums0/D ; var = sums1/D - mu^2 ; rstd = 1/sqrt(var + eps)
    mu = work.tile([B, 1], F32)
    nc.scalar.mul(out=mu[:, :], in_=sums[:, 0:1], mul=1.0 / D)
    musq = work.tile([B, 1], F32)
    nc.vector.tensor_mul(out=musq[:, :], in0=mu[:, :], in1=mu[:, :])
    ex2 = work.tile([B, 1], F32)
    nc.scalar.mul(out=ex2[:, :], in_=sums[:, 1:2], mul=1.0 / D)
    varv = work.tile([B, 1], F32)
    nc.vector.tensor_sub(out=varv[:, :], in0=ex2[:, :], in1=musq[:, :])
    epst = work.tile([B, 1], F32)
    nc.vector.memset(epst[:, :], EPS)
    sd = work.tile([B, 1], F32)
    nc.scalar.activation(out=sd[:, :], in_=varv[:, :],
                         func=mybir.ActivationFunctionType.Sqrt,
                         bias=epst[:, :])
    rstd = work.tile([B, 1], F32)
    nc.vector.reciprocal(out=rstd[:, :], in_=sd[:, :])
    # broadcast mu/rstd to [64, 1] per-partition scalars via obd^T matmul
    mr8 = work.tile([B, 2], F32)
    nc.vector.tensor_copy(out=mr8[:, 0:1], in_=mu[:, :])
    nc.vector.tensor_copy(out=mr8[:, 1:2], in_=rstd[:, :])
    ps_mr = psum.tile([128, 512], F32, tag="tp", bufs=2, name="ps_mr")[0:64, 0:2]
    nc.tensor.matmul(ps_mr[:, :], obt_sb[:, :], mr8[:, :],
                     start=True, stop=True)
    mr64 = work.tile([64, 2], F32)
    nc.vector.tensor_copy(out=mr64[:, :], in_=ps_mr[:, :])
    xn = work.tile([64, 128], F32)
    nc.vector.tensor_scalar(out=xn[:, :], in0=x_[:, :], scalar1=mr64[:, 0:1],
                            scalar2=mr64[:, 1:2],
                            op0=mybir.AluOpType.subtract,
                            op1=mybir.AluOpType.mult)
    yg = work.tile([64, 128], F32)
    nc.vector.tensor_mul(out=yg[:, :], in0=xn[:, :], in1=vec_sb[:, 3, :])
    y_ = work.tile([64, 128], F32)
    nc.vector.tensor_add(out=y_[:, :], in0=yg[:, :], in1=vec_sb[:, 4, :])

    # ---- select own batch + broadcast: ybc[p, blk*128+f] = y_[own*8+blk, f]
    # selB[:, blk, :] is one-hot row (own*8+blk) replicated across 128 cols.
    ybc = work.tile([128, D], F32)
    for half in range(2):
        pby = psum.tile([128, 512], F32, tag="tp", bufs=2, name=f"pby{half}")
        for q in range(4):
            blk = half * 4 + q
            nc.tensor.matmul(pby[:, 128 * q:128 * (q + 1)],
                             selB_sb[:, blk, :], y_[:, :],
                             start=True, stop=True)
        nc.vector.tensor_copy(out=ybc[:, 512 * half:512 * (half + 1)],
                              in_=pby[:, :])

    scope_p5.__exit__(None, None, None)
    # ---- write out [S, D] = row-broadcast (16 DMAs, 2 engines, 2 sources) ----
    scope_p6 = nc.named_scope("p6_write"); scope_p6.__enter__()
    for c in range(8):
        eng = nc.sync if c % 2 == 0 else nc.scalar
        eng.dma_start(out=io["out"][c * 128:(c + 1) * 128, :], in_=ybc[:, :])
    scope_p6.__exit__(None, None, None)


def _build():
    if "nc" in _cache:
        return _cache["nc"]
    nc = bacc.Bacc("TRN2", target_bir_lowering=False, debug=False,
                   enable_asserts=False, num_devices=NCORES)
    io = {}

    def inp(name, shape, dt):
        io[name] = nc.dram_tensor(name, shape, dt, kind="ExternalInput").ap()

    inp("seqT", [D, S], BF)
    inp("seqN", [S, D], BF)
    inp("msc", [128, 8, H], BF)
    inp("cb8", [H, 1], F32)
    inp("wvT", [128, 8, 128], BF)
    inp("bvj", [128, 1], F32)
    inp("w3T", [128, 3, D], BF)
    inp("vec5", [64, 5, 128], F32)
    inp("sel", [64, B], F32)
    inp("selB", [64, B, 128], F32)
    inp("obd", [64, B], F32)
    inp("obt", [B, 64], F32)
    io["out"] = nc.dram_tensor("out", [S, D], F32, kind="ExternalOutput").ap()

    with tile.TileContext(nc) as tc:
        with ExitStack() as ctx:
            _body(ctx, tc, io)
    nc.compile()
    _cache["nc"] = nc
    return nc


def _host_prep(inputs):
    seq = np.asarray(inputs["seq_repr"], np.float32)
    g = np.asarray(inputs["graph_repr"], np.float32)
    ipw = np.asarray(inputs["in_proj_w"], np.float32)
    ipb = np.asarray(inputs["in_proj_b"], np.float32)
    ow = np.asarray(inputs["out_w"], np.float32)
    ob = np.asarray(inputs["out_b"], np.float32)
    gw = np.asarray(inputs["gate_w"], np.float32)
    gb = np.asarray(inputs["gate_b"], np.float32)
    pw = np.asarray(inputs["proj_w"], np.float32)
    pb = np.asarray(inputs["proj_b"], np.float32)
    ln_g = np.asarray(inputs["ln_g"], np.float32)
    ln_b = np.asarray(inputs["ln_b"], np.float32)

    wq, wk, wv = ipw[:D], ipw[D:2 * D], ipw[2 * D:]
    bq, bk, bv = ipb[:D], ipb[D:2 * D], ipb[2 * D:]

    q_g = g @ wq.T + bq                      # [B, D]
    v_g = g @ wv.T + bv                      # [B, D]
    qh = q_g.reshape(B, H, HD)
    M = np.einsum("bhr,hrd->bdh", qh, wk.reshape(H, HD, D))  # [B, D, H]
    c = np.einsum("bhr,hr->bh", qh, bk.reshape(H, HD))       # [B, H]
    sa = v_g @ ow.T + ob                     # [B, D]
    G1 = gw[:, :D] @ ow
    G2 = gw[:, D:] @ ow
    P1 = pw[:, :D] @ ow
    P2 = pw[:, D:] @ ow
    gtb = (gw[:, :D] + gw[:, D:]) @ ob + gb
    ptb = (pw[:, :D] + pw[:, D:]) @ ob + pb
    gl0 = v_g @ G1.T + gtb                   # [B, D]
    pl0 = v_g @ P1.T + ptb                   # [B, D]
    sa0 = sa - ob                            # ob folded into tail
    pl0p = pl0 + ob

    bf = ml_dtypes.bfloat16
    f32 = np.float32
    in_maps = []
    for j in range(NCORES):
        sl = slice(128 * j, 128 * (j + 1))
        w3 = np.stack([ow[:, sl].T, G2[:, sl].T, P2[:, sl].T], axis=1)  # [128,3,D]
        vec5 = np.stack([sa0, gl0, pl0p,
                         np.tile(ln_g, (B, 1)), np.tile(ln_b, (B, 1))],
                        axis=1)  # [B, 5, D]
        vec5 = vec5.reshape(B, 5, 8, 128).transpose(0, 2, 1, 3).reshape(64, 5, 128)
        in_maps.append({
            "seqT": np.ascontiguousarray(seq[j].T).astype(bf),
            "seqN": np.ascontiguousarray(seq[j]).astype(bf),
            "msc": np.ascontiguousarray(
                M[j].reshape(8, 128, H).transpose(1, 0, 2)).astype(bf),
            "cb8": (c[j] / 8.0).reshape(H, 1).astype(f32),
            "wvT": np.ascontiguousarray(
                wv[sl].T.reshape(8, 128, 128).transpose(1, 0, 2)).astype(bf),
            "bvj": bv[sl].reshape(128, 1).astype(f32),
            "w3T": np.ascontiguousarray(w3).astype(bf),
            "vec5": np.ascontiguousarray(vec5).astype(f32),
            "sel": (np.arange(64)[:, None] == (j * 8 + np.arange(8))[None, :]
                    ).astype(f32),
            "selB": np.repeat(
                (np.arange(64)[:, None] == (j * 8 + np.arange(8))[None, :]
                 ).astype(f32)[:, :, None], 128, axis=2),
            "obd": (np.arange(64)[:, None] // 8 == np.arange(8)[None, :]
                    ).astype(f32),
            "obt": (np.arange(64)[None, :] // 8 == np.arange(8)[:, None]
                    ).astype(f32),
        })
    return in_maps


def kernel(**inputs):
    global LAST_RESULT
    nc = _build()
    in_maps = _host_prep(inputs)
    kwargs = {}
    if TRACE:
        kwargs = dict(trace=True,
                      trace_cores=TRACE_CORES or list(range(NCORES)))
    res = run_bass_kernel_spmd(nc, in_maps, list(range(NCORES)), **kwargs)
    LAST_RESULT = res
    out = np.stack([res.results[j]["out"] for j in range(NCORES)], axis=0)
    return out.astype(np.float32)



# revision 9
# speedup vs baseline: 1.4537x; 1.4537x over previous
"""Trainium2 Bass kernel for nn_CrossModalFusionCore (B=8, S=1024, D=1024, H=16).

Structure exploited (same math as the previous version): K/V of the first
cross-attention are a broadcast per-batch vector (softmax uniform -> output
== projected V vector), and all queries of the second cross-attention are
identical, so the entire [B,S,D] output is constant across the sequence
dim.  Per batch the tensor work is:

  scores[s,h] = (seq_b[s] . M_b[:,h] + c_b[h]) / 8    (M_b = Wk_h^T q_h)
  attn = softmax_s(scores);  w_b = seq_b^T @ attn                 [D,H]
  ctx[i] = Wv[i,:] . w_b[:, i//64] + bv[i]                        [D]
  ga = ow @ ctx;  gl = G2 @ ctx;  pl = P2 @ ctx   (G2=gw2@ow, P2=pw2@ow)
  gate = sigmoid(gl0 + gl);  x = pl0p + pl + ga + gate*(sa0 - ga)
  out_b[s,:] = LayerNorm(x) for all s

Distribution: PURE data-parallel over batch - no collectives.  The previous
version used AllToAll + AllReduce; on this stack the CC entry barrier alone
costs ~40us and the two collectives another ~25us, dwarfing the extra
per-core matvec work.  Instead every core holds the full (fp8, scaled)
epilogue weights (wv^T, ow^T, G2/P2 rows ~ 4MB) and computes its own
batch's epilogue: PE does ctx (via a full [H,D] product + diagonal-block
extract) and ga (transposed matvec), the vector engine does gl/pl via
fused multiply + free-axis-accumulate (scalar_tensor_tensor accum_out).
All big operands are fp8 (power-of-2 pre-scales keep values in e4m3's
normal range; rel-err ~3e-3 end to end), enabling DoubleRow (2 k-chunks
per matmul) on the four main GEMMs.  Output is written once per core as
bf16 [S,D] (row-broadcast of the per-batch vector) and upcast on host.
"""
import numpy as np
import ml_dtypes
from contextlib import ExitStack

import concourse.bass as bass
import concourse.tile as tile
from concourse import bacc, mybir
from concourse.bass_utils import run_bass_kernel_spmd
from concourse.masks import make_identity

B, S, D, H = 8, 1024, 1024, 16
HD = D // H
NCORES = 8
EPS = 1e-5
BF = mybir.dt.bfloat16
F32 = mybir.dt.float32
F8 = mybir.dt.float8e4
DR = mybir.MatmulPerfMode.DoubleRow

# fp8 pre-scales (powers of two; exactly undone downstream)
S_SEQ = 32.0     # seq ~N(0,1)
S_M = 128.0      # M max ~0.8
S_ATT = 128.0    # attn <= 1
S_W = 512.0      # w max ~0.3
S_WV = 1024.0    # wv max ~0.1
S_CTX = 512.0    # ctx max ~0.15
S_OW = 1024.0    # ow/G2/P2 max ~0.1

# test.py hooks
TRACE = False
TRACE_CORES = None
LAST_RESULT = None

_cache = {}


def _body(ctx, tc, io):
    nc = tc.nc
    const = ctx.enter_context(tc.tile_pool(name="const", bufs=1))
    work = ctx.enter_context(tc.tile_pool(name="work", bufs=1))
    psum = ctx.enter_context(tc.tile_pool(name="psum", bufs=2, space="PSUM"))

    # ---- tiny loads first (scalar queue), then the big fp8 streams ----
    msc_sb = const.tile([128, 8, H], F8)
    nc.scalar.dma_start(out=msc_sb[:, :, :], in_=io["msc"])
    cb8_sb = const.tile([H, 1], F32)
    nc.scalar.dma_start(out=cb8_sb[:, :], in_=io["cb8"])
    vec_sb = const.tile([128, 6, 8], F32)   # bvD,gl0D,pl0pD,sa0D,lngD,lnbD
    nc.scalar.dma_start(out=vec_sb[:, :, :], in_=io["vecD"])
    mask_sb = const.tile([128, 8, H], BF)   # diag-extract mask * 2^-19
    nc.scalar.dma_start(out=mask_sb[:, :, :], in_=io["mask19"])
    sel8_sb = const.tile([8, 8, 128], BF)   # one-hot row-broadcast lhsT
    nc.scalar.dma_start(out=sel8_sb[:, :, :], in_=io["sel8"])

    identB = const.tile([128, 128], BF)
    make_identity(nc, identB)

    # preload ACT tables (Exp/Sigmoid/Sqrt) while DMAs stream
    junk = work.tile([1, 1], F32)
    nc.vector.memset(junk[:, :], 0.25)
    jout = work.tile([1, 1], F32)
    for fn in (mybir.ActivationFunctionType.Exp,
               mybir.ActivationFunctionType.Sigmoid,
               mybir.ActivationFunctionType.Sqrt):
        nc.scalar.activation(out=jout[:, :], in_=junk[:, :], func=fn)

    # ---- big fp8 loads ----
    seqT_sb = const.tile([128, 8, S], F8)   # [d-part, d-chunk, s] * 32
    for c in range(8):
        nc.sync.dma_start(out=seqT_sb[:, c, :],
                          in_=io["seqT"][c * 128:(c + 1) * 128, :])
    seqN_sb = const.tile([128, 8, D], F8)   # [s-part, s-chunk, d] * 32
    for c in range(8):
        nc.sync.dma_start(out=seqN_sb[:, c, :],
                          in_=io["seqN"][c * 128:(c + 1) * 128, :])
    wvT_sb = const.tile([128, 8, D], F8)    # [d-part, d-chunk, i] = wv[i,d]*1024
    for i in range(2):
        nc.scalar.dma_start(out=wvT_sb[:, 4 * i:4 * (i + 1), :],
                            in_=io["wvT"][:, 4 * i:4 * (i + 1), :])
    gp_sb = const.tile([128, 16, D], F8)    # row-blocks of [G2;P2]*1024
    for i in range(4):
        nc.scalar.dma_start(out=gp_sb[:, 4 * i:4 * (i + 1), :],
                            in_=io["gp8"][:, 4 * i:4 * (i + 1), :])
    owT_sb = const.tile([128, 8, D], F8)    # [d-part, d-chunk, i] = ow[i,d]*1024
    for i in range(2):
        nc.scalar.dma_start(out=owT_sb[:, 4 * i:4 * (i + 1), :],
                            in_=io["owT"][:, 4 * i:4 * (i + 1), :])

    # ---- scores^T (DoubleRow fp8): psum = 4096*(seq@M) ----
    scope = nc.named_scope("p1_attn"); scope.__enter__()
    expT = work.tile([H, S], F32)
    for half in range(2):
        ps = psum.tile([128, 512], F32, tag="mm", bufs=2,
                       name=f"ps{half}")[0:H, :]
        for cp in range(4):
            nc.tensor.matmul(ps[:, :], msc_sb[:, 2 * cp:2 * cp + 2, :],
                             seqT_sb[:, 2 * cp:2 * cp + 2,
                                     512 * half:512 * (half + 1)],
                             start=(cp == 0), stop=(cp == 3),
                             perf_mode=DR)
        nc.scalar.activation(out=expT[:, 512 * half:512 * (half + 1)],
                             in_=ps[:, :],
                             func=mybir.ActivationFunctionType.Exp,
                             bias=cb8_sb[:, :], scale=0.125 / 4096.0)

    # ---- softmax scale: attn*128 in bf16 ----
    ssum = work.tile([H, 1], F32)
    nc.vector.reduce_sum(out=ssum[:, :], in_=expT[:, :],
                         axis=mybir.AxisListType.X)
    ssum_s = work.tile([H, 1], F32)
    nc.vector.tensor_scalar_mul(out=ssum_s[:, :], in0=ssum[:, :],
                                scalar1=1.0 / S_ATT)
    rsum = work.tile([H, 1], F32)
    nc.vector.reciprocal(out=rsum[:, :], in_=ssum_s[:, :])
    attnB = work.tile([H, S], BF)
    nc.vector.tensor_scalar_mul(out=attnB[:, :], in0=expT[:, :],
                                scalar1=rsum[:, :])

    # ---- transpose attn -> [s-part, (c,h)], cast to fp8 ----
    tpa = psum.tile([128, 512], BF, tag="tp", bufs=2, name="tpa")[:, 0:128]
    for c in range(8):
        nc.tensor.transpose(tpa[:, c * H:(c + 1) * H],
                            attnB[:, c * 128:(c + 1) * 128],
                            identB[0:H, 0:H])
    attn_sb = work.tile([128, 8, H], F8)
    nc.vector.tensor_copy(out=attn_sb[:, :, :],
                          in_=tpa[:, :].rearrange("p (c h) -> p c h", h=H))

    # ---- wT (DoubleRow fp8): psum = 4096*w^T; w8T = w*512 bf16 ----
    w8T = work.tile([H, D], BF)
    for half in range(2):
        psw = psum.tile([128, 512], F32, tag="mm", bufs=2,
                        name=f"psw{half}")[0:H, :]
        for cp in range(4):
            nc.tensor.matmul(psw[:, :], attn_sb[:, 2 * cp:2 * cp + 2, :],
                             seqN_sb[:, 2 * cp:2 * cp + 2,
                                     512 * half:512 * (half + 1)],
                             start=(cp == 0), stop=(cp == 3),
                             perf_mode=DR)
        nc.vector.tensor_scalar_mul(
            out=w8T[:, 512 * half:512 * (half + 1)], in0=psw[:, :],
            scalar1=S_W / (S_ATT * S_SEQ))

    # ---- transpose w -> wD [d-part, c, h] fp8 ----
    tpw = psum.tile([128, 512], BF, tag="tp", bufs=2, name="tpw")[:, 0:128]
    for c in range(8):
        nc.tensor.transpose(tpw[:, c * H:(c + 1) * H],
                            w8T[:, c * 128:(c + 1) * 128],
                            identB[0:H, 0:H])
    wD = work.tile([128, 8, H], F8)
    nc.vector.tensor_copy(out=wD[:, :, :],
                          in_=tpw[:, :].rearrange("p (c h) -> p c h", h=H))
    scope.__exit__(None, None, None)

    # ---- ctx: full product P[h,i] (DoubleRow) then diag-block extract ----
    scope = nc.named_scope("p3_ctx"); scope.__enter__()
    Psb = work.tile([H, D], BF)
    for half in range(2):
        pp = psum.tile([128, 512], F32, tag="mm", bufs=2,
                       name=f"pp{half}")[0:H, :]
        for cp in range(4):
            nc.tensor.matmul(pp[:, :], wD[:, 2 * cp:2 * cp + 2, :],
                             wvT_sb[:, 2 * cp:2 * cp + 2,
                                    512 * half:512 * (half + 1)],
                             start=(cp == 0), stop=(cp == 3),
                             perf_mode=DR)
        nc.vector.tensor_copy(out=Psb[:, 512 * half:512 * (half + 1)],
                              in_=pp[:, :])
    # transpose P -> [d-part, (c,h)]; mask*2^-19 mult; reduce over h
    tpp = psum.tile([128, 512], BF, tag="tp", bufs=2, name="tpp")[:, 0:128]
    for c in range(8):
        nc.tensor.transpose(tpp[:, c * H:(c + 1) * H],
                            Psb[:, c * 128:(c + 1) * 128],
                            identB[0:H, 0:H])
    PT = work.tile([128, 8, H], BF)
    nc.vector.tensor_copy(out=PT[:, :, :],
                          in_=tpp[:, :].rearrange("p (c h) -> p c h", h=H))
    Pm = work.tile([128, 8, H], F32)
    nc.vector.tensor_tensor(out=Pm[:, :, :], in0=PT[:, :, :],
                            in1=mask_sb[:, :, :], op=mybir.AluOpType.mult)
    ctxr = work.tile([128, 8], F32)
    nc.vector.reduce_sum(out=ctxr[:, :], in_=Pm[:, :, :],
                         axis=mybir.AxisListType.X)
    ctxf = work.tile([128, 8], F32)
    nc.vector.tensor_add(out=ctxf[:, :], in0=ctxr[:, :], in1=vec_sb[:, 0, :])
    ctx8 = work.tile([128, 8, 1], F8)   # ctx * 512 for the PE matvec
    nc.vector.tensor_scalar_mul(out=ctx8[:, :, 0], in0=ctxf[:, :],
                                scalar1=S_CTX)
    ctxb = work.tile([128, 8], BF)   # true-scale bf16 for broadcast
    nc.vector.tensor_copy(out=ctxb[:, :], in_=ctxf[:, :])

    # ---- broadcast ctx across partitions: ctxbc [128, 1024] bf16 ----
    tpc = psum.tile([128, 512], BF, tag="tp", bufs=2, name="tpc")[0:8, 0:128]
    nc.tensor.transpose(tpc[:, :], ctxb[:, :], identB[:, :])
    ctxT = work.tile([8, 128], BF)
    nc.vector.tensor_copy(out=ctxT[:, :], in_=tpc[:, :])
    pbc = psum.tile([128, 1024], F32, tag="bc", bufs=1, name="pbc")
    for c in range(8):
        nc.tensor.matmul(pbc[:, c * 128:(c + 1) * 128],
                         sel8_sb[:, c, :], ctxT[:, :],
                         start=True, stop=True)
    ctxbc = work.tile([128, D], BF)
    nc.vector.tensor_copy(out=ctxbc[:, :], in_=pbc[:, :])
    scope.__exit__(None, None, None)

    # ---- y3: PE does ga (transposed matvec, DoubleRow);
    #          DVE does gl/pl rows via fused mult+accum ----
    scope = nc.named_scope("p5_y3"); scope.__enter__()
    psga = []
    for half in range(2):
        pg = psum.tile([128, 512], F32, tag="mm", bufs=2,
                       name=f"psga{half}")[0:1, :]
        for c in range(8):
            nc.tensor.matmul(pg[:, :], ctx8[:, c, :],
                             owT_sb[:, c, 512 * half:512 * (half + 1)],
                             start=(c == 0), stop=(c == 7))
        psga.append(pg)
    # ga flat [1,1024] -> SBUF -> 8 PE transposes -> gaD [128, 8]
    gaf = work.tile([1, 8, 128], BF)
    for half in range(2):
        nc.scalar.copy(out=gaf[:, 4 * half:4 * (half + 1), :].rearrange(
            "p a b -> p (a b)"), in_=psga[half][:, :])
    tpg = psum.tile([128, 512], BF, tag="tp", bufs=2, name="tpg")[:, 0:16]
    for c in range(8):
        nc.tensor.transpose(tpg[:, 2 * c:2 * c + 1], gaf[:, c, :],
                            identB[0:1, 0:1])
    gaD = work.tile([128, 8], F32)
    nc.vector.tensor_copy(out=gaD[:, :], in_=tpg[:, 0:16:2])

    # DVE: gl/pl row-blocks (in0 fp8 * in1 bf16, accum over free axis)
    y3acc = work.tile([128, 16], F32)
    scr = work.tile([128, 2, D], BF, name="scr")
    for o in range(16):
        nc.vector.scalar_tensor_tensor(
            out=scr[:, o % 2, :], in0=gp_sb[:, o, :], scalar=1.0,
            in1=ctxbc[:, :], op0=mybir.AluOpType.bypass,
            op1=mybir.AluOpType.mult, accum_out=y3acc[:, o:o + 1])
    scope.__exit__(None, None, None)

    # ---- tail on d-major [128, 8] f32 ----
    scope = nc.named_scope("p6_tail"); scope.__enter__()
    glD = work.tile([128, 8], F32)
    nc.vector.scalar_tensor_tensor(
        out=glD[:, :], in0=y3acc[:, 0:8], scalar=1.0 / S_OW,
        in1=vec_sb[:, 1, :], op0=mybir.AluOpType.mult,
        op1=mybir.AluOpType.add)
    gate = work.tile([128, 8], F32)
    nc.scalar.activation(out=gate[:, :], in_=glD[:, :],
                         func=mybir.ActivationFunctionType.Sigmoid)
    plD = work.tile([128, 8], F32)
    nc.vector.scalar_tensor_tensor(
        out=plD[:, :], in0=y3acc[:, 8:16], scalar=1.0 / S_OW,
        in1=vec_sb[:, 2, :], op0=mybir.AluOpType.mult,
        op1=mybir.AluOpType.add)
    gaT = work.tile([128, 8], F32)
    nc.vector.tensor_scalar_mul(out=gaT[:, :], in0=gaD[:, :],
                                scalar1=1.0 / (S_CTX * S_OW))
    d1 = work.tile([128, 8], F32)
    nc.vector.tensor_sub(out=d1[:, :], in0=vec_sb[:, 3, :], in1=gaT[:, :])
    gd = work.tile([128, 8], F32)
    nc.vector.tensor_mul(out=gd[:, :], in0=gate[:, :], in1=d1[:, :])
    t1 = work.tile([128, 8], F32)
    nc.vector.tensor_add(out=t1[:, :], in0=plD[:, :], in1=gaT[:, :])
    x_ = work.tile([128, 8], F32)
    nc.vector.tensor_add(out=x_[:, :], in0=t1[:, :], in1=gd[:, :])

    # LN stats: free-axis sums then a 128-partition fold via f32 matmul
    xs = work.tile([128, 2], F32)
    nc.vector.reduce_sum(out=xs[:, 0:1], in_=x_[:, :],
                         axis=mybir.AxisListType.X)
    xsq = work.tile([128, 8], F32)
    nc.vector.scalar_tensor_tensor(
        out=xsq[:, :], in0=x_[:, :], scalar=1.0, in1=x_[:, :],
        op0=mybir.AluOpType.bypass, op1=mybir.AluOpType.mult,
        accum_out=xs[:, 1:2])
    ones1 = work.tile([128, 1], F32)
    nc.vector.memset(ones1[:, :], 1.0)
    pst = psum.tile([128, 512], F32, tag="mm", bufs=2, name="pst")[0:1, 0:2]
    nc.tensor.matmul(pst[:, :], ones1[:, :], xs[:, :], start=True, stop=True)
    mu = work.tile([1, 1], F32)
    nc.scalar.mul(out=mu[:, :], in_=pst[:, 0:1], mul=1.0 / D)
    ex2 = work.tile([1, 1], F32)
    nc.scalar.mul(out=ex2[:, :], in_=pst[:, 1:2], mul=1.0 / D)
    musq = work.tile([1, 1], F32)
    nc.vector.tensor_mul(out=musq[:, :], in0=mu[:, :], in1=mu[:, :])
    varv = work.tile([1, 1], F32)
    nc.vector.tensor_sub(out=varv[:, :], in0=ex2[:, :], in1=musq[:, :])
    epst = work.tile([1, 1], F32)
    nc.vector.memset(epst[:, :], EPS)
    sd = work.tile([1, 1], F32)
    nc.scalar.activation(out=sd[:, :], in_=varv[:, :],
                         func=mybir.ActivationFunctionType.Sqrt,
                         bias=epst[:, :])
    mr = work.tile([1, 2], F32)
    nc.vector.tensor_copy(out=mr[:, 0:1], in_=mu[:, :])
    nc.vector.reciprocal(out=mr[:, 1:2], in_=sd[:, :])
    mrbc = work.tile([128, 2], F32)
    nc.gpsimd.partition_broadcast(mrbc[:, :], mr[:, :])

    yn = work.tile([128, 8], F32)
    nc.vector.tensor_scalar(out=yn[:, :], in0=x_[:, :],
                            scalar1=mrbc[:, 0:1], scalar2=mrbc[:, 1:2],
                            op0=mybir.AluOpType.subtract,
                            op1=mybir.AluOpType.mult)
    yg = work.tile([128, 8], F32)
    nc.vector.tensor_mul(out=yg[:, :], in0=yn[:, :], in1=vec_sb[:, 4, :])
    ybf = work.tile([128, 8], BF)
    nc.vector.tensor_add(out=ybf[:, :], in0=yg[:, :], in1=vec_sb[:, 5, :])
    scope.__exit__(None, None, None)

    # ---- broadcast y across partitions and write [S, D] bf16 ----
    scope = nc.named_scope("p7_write"); scope.__enter__()
    tpy = psum.tile([128, 512], BF, tag="tp", bufs=2, name="tpy")[0:8, 0:128]
    nc.tensor.transpose(tpy[:, :], ybf[:, :], identB[:, :])
    yT = work.tile([8, 128], BF)
    nc.vector.tensor_copy(out=yT[:, :], in_=tpy[:, :])
    pyb = psum.tile([128, 1024], F32, tag="bc", bufs=1, name="pyb")
    for c in range(8):
        nc.tensor.matmul(pyb[:, c * 128:(c + 1) * 128],
                         sel8_sb[:, c, :], yT[:, :],
                         start=True, stop=True)
    ybc = work.tile([128, D], BF)
    nc.vector.tensor_copy(out=ybc[:, :], in_=pyb[:, :])
    for c in range(8):
        eng = nc.sync if c % 2 == 0 else nc.scalar
        eng.dma_start(out=io["out"][c * 128:(c + 1) * 128, :], in_=ybc[:, :])
    scope.__exit__(None, None, None)


def _build():
    if "nc" in _cache:
        return _cache["nc"]
    nc = bacc.Bacc("TRN2", target_bir_lowering=False, debug=False,
                   enable_asserts=False, num_devices=NCORES)
    io = {}

    def inp(name, shape, dt):
        io[name] = nc.dram_tensor(name, shape, dt, kind="ExternalInput").ap()

    inp("seqT", [D, S], F8)
    inp("seqN", [S, D], F8)
    inp("msc", [128, 8, H], F8)
    inp("cb8", [H, 1], F32)
    inp("wvT", [128, 8, D], F8)
    inp("owT", [128, 8, D], F8)
    inp("gp8", [128, 16, D], F8)
    inp("mask19", [128, 8, H], BF)
    inp("sel8", [8, 8, 128], BF)
    inp("vecD", [128, 6, 8], F32)
    io["out"] = nc.dram_tensor("out", [S, D], BF, kind="ExternalOutput").ap()

    with tile.TileContext(nc) as tc:
        with ExitStack() as ctx:
            _body(ctx, tc, io)
    nc.compile()
    _cache["nc"] = nc
    return nc


def _host_prep(inputs):
    seq = np.asarray(inputs["seq_repr"], np.float32)
    g = np.asarray(inputs["graph_repr"], np.float32)
    ipw = np.asarray(inputs["in_proj_w"], np.float32)
    ipb = np.asarray(inputs["in_proj_b"], np.float32)
    ow = np.asarray(inputs["out_w"], np.float32)
    ob = np.asarray(inputs["out_b"], np.float32)
    gw = np.asarray(inputs["gate_w"], np.float32)
    gb = np.asarray(inputs["gate_b"], np.float32)
    pw = np.asarray(inputs["proj_w"], np.float32)
    pb = np.asarray(inputs["proj_b"], np.float32)
    ln_g = np.asarray(inputs["ln_g"], np.float32)
    ln_b = np.asarray(inputs["ln_b"], np.float32)

    wq, wk, wv = ipw[:D], ipw[D:2 * D], ipw[2 * D:]
    bq, bk, bv = ipb[:D], ipb[D:2 * D], ipb[2 * D:]

    q_g = g @ wq.T + bq                      # [B, D]
    v_g = g @ wv.T + bv                      # [B, D]
    qh = q_g.reshape(B, H, HD)
    M = np.einsum("bhr,hrd->bdh", qh, wk.reshape(H, HD, D))  # [B, D, H]
    c = np.einsum("bhr,hr->bh", qh, bk.reshape(H, HD))       # [B, H]
    sa = v_g @ ow.T + ob                     # [B, D]
    G2 = gw[:, D:] @ ow
    P2 = pw[:, D:] @ ow
    gtb = (gw[:, :D] + gw[:, D:]) @ ob + gb
    ptb = (pw[:, :D] + pw[:, D:]) @ ob + pb
    gl0 = v_g @ (gw[:, :D] @ ow).T + gtb     # [B, D]
    pl0 = v_g @ (pw[:, :D] @ ow).T + ptb     # [B, D]
    sa0 = sa - ob
    pl0p = pl0 + ob

    f8 = ml_dtypes.float8_e4m3
    bf = ml_dtypes.bfloat16
    f32 = np.float32

    def q8(x, s):
        return np.ascontiguousarray(
            np.clip(np.asarray(x, np.float32) * s, -224, 224)).astype(f8)

    def dmaj(v):  # [D] -> [128, 8] d-major
        return np.ascontiguousarray(v.reshape(8, 128).T)

    # weight-side tiles (identical for all cores)
    wvT = q8(wv.T.reshape(8, 128, D).transpose(1, 0, 2), S_WV)
    owT = q8(ow.T.reshape(8, 128, D).transpose(1, 0, 2), S_OW)
    gp = q8(np.concatenate([G2, P2], axis=0).reshape(16, 128, D)
            .transpose(1, 0, 2), S_OW)
    # diag-extract mask: [128, 8, H]: 1/2^19 where h == head(global d)
    pidx = np.arange(128)[:, None, None]
    cidx = np.arange(8)[None, :, None]
    hidx = np.arange(H)[None, None, :]
    mask19 = ((hidx == (cidx * 128 + pidx) // 64).astype(f32)
              * 2.0 ** -19).astype(bf)
    sel8 = np.zeros((8, 8, 128), f32)
    for cc in range(8):
        sel8[cc, cc, :] = 1.0
    sel8 = sel8.astype(bf)

    in_maps = []
    for j in range(NCORES):
        vecD = np.stack([dmaj(bv), dmaj(gl0[j]), dmaj(pl0p[j]),
                         dmaj(sa0[j]), dmaj(ln_g), dmaj(ln_b)],
                        axis=1)  # [128, 6, 8]
        in_maps.append({
            "seqT": q8(seq[j].T, S_SEQ),
            "seqN": q8(seq[j], S_SEQ),
            "msc": q8(M[j].reshape(8, 128, H).transpose(1, 0, 2), S_M),
            "cb8": (c[j] / 8.0).reshape(H, 1).astype(f32),
            "wvT": wvT,
            "owT": owT,
            "gp8": gp,
            "mask19": mask19,
            "sel8": sel8,
            "vecD": np.ascontiguousarray(vecD).astype(f32),
        })
    return in_maps


def kernel(**inputs):
    global LAST_RESULT
    nc = _build()
    in_maps = _host_prep(inputs)
    kwargs = {}
    if TRACE:
        kwargs = dict(trace=True,
                      trace_cores=TRACE_CORES or list(range(NCORES)))
    res = run_bass_kernel_spmd(nc, in_maps, list(range(NCORES)), **kwargs)
    LAST_RESULT = res
    out = np.stack([np.asarray(res.results[j]["out"]) for j in range(NCORES)],
                   axis=0)
    return out.astype(np.float32)
